# revision 1
# baseline (speedup 1.0000x reference)
"""Trainium2 Bass kernel for nn_BasicBlock (ReActNet-style binary basic block).

Forward math (stop_gradient collapses in forward):
    s1   = sign(x + b11)                          in {-1, 0, +1}
    y1   = conv3x3(s1, sign(w1)) * (scale1*inv1) + (be1 - m1*inv1)   [BN folded]
    pre1 = y1 + x + b12
    p1   = prelu(pre1, a1)
    out1 = p1 + b13                               [b13 folded downstream]
    s2   = sign(out1 + b21) = sign(p1 + (b13+b21))
    y2   = conv1x1(s2, sign(w2)) * (scale2*inv2) + (be2 - m2*inv2)
    pre2 = y2 + out1 + b22 = y2 + p1 + (b22 + b13)
    out2 = prelu(pre2, a2) + b23

Both convs are +-1 x +-1 matmuls: exact in bf16 with fp32 PSUM accumulation.
conv3x3 = 9 shifted matmuls over a zero-padded 34x34 tile.
Sharding: data-parallel over batch, 16 images per core on 8 cores.
"""

import numpy as np
import ml_dtypes

import concourse.bacc as bacc
import concourse.mybir as mybir
from concourse import tile
from concourse.bass_utils import run_bass_kernel_spmd

N_CORES = 8
B, C, H, W = 128, 256, 32, 32
PIMG = B // N_CORES  # images per core
EPS = 1e-5

_CACHE = {}


def _build_program(n_img):
    """Build the SPMD per-core Bass/Tile program (same program on all cores)."""
    f32, bf16 = mybir.dt.float32, mybir.dt.bfloat16
    AF = mybir.ActivationFunctionType
    ALU = mybir.AluOpType

    nc = bacc.Bacc("TRN2", target_bir_lowering=False, debug=False,
                   num_devices=N_CORES)

    x_d = nc.dram_tensor("x", [n_img, C, H, W], f32, kind="ExternalInput").ap()
    w1_d = nc.dram_tensor("w1t", [2, 128, 9, 2, 128], bf16,
                          kind="ExternalInput").ap()
    w2_d = nc.dram_tensor("w2t", [2, 128, 2, 128], bf16,
                          kind="ExternalInput").ap()
    cv_d = nc.dram_tensor("cv", [2, 128, 11], f32, kind="ExternalInput").ap()
    out_d = nc.dram_tensor("out", [n_img, C, H, W], f32,
                           kind="ExternalOutput").ap()

    # cv columns: 0 b11, 1 A1, 2 C1, 3 beta1, 4 alpha1, 5 s2bias,
    #             6 A2, 7 C2p, 8 beta2, 9 alpha2, 10 b23
    with tile.TileContext(nc) as tc:
        with tc.tile_pool(name="wp", bufs=1) as wp, \
             tc.tile_pool(name="work", bufs=1) as work, \
             tc.tile_pool(name="ps", bufs=1, space="PSUM") as ps:

            # consts first (tiny, ONE dma) so sign1(0) starts while weights
            # stream
            cvt = wp.tile([128, 2, 11], f32, name="cvt")
            nc.sync.dma_start(out=cvt,
                              in_=cv_d.rearrange("t p c -> p t c"))
            cv = [cvt[:, 0], cvt[:, 1]]

            w1sb, w2sb = [], []

            def load_weights():
                # emitted after stage_a(0): x(0)+sign1(0) overlap the 1.4MB
                # weight stream; per-oct split lets conv1(0) oct=0 start as
                # soon as its half of w1 lands
                for t in range(2):
                    w1t_ = wp.tile([128, 9, 2, 128], bf16, name=f"w1sb{t}")
                    for oc in range(2):
                        nc.sync.dma_start(out=w1t_[:, :, oc, :],
                                          in_=w1_d[t][:, :, oc, :])
                    w1sb.append(w1t_)
                for t in range(2):
                    w2t_ = wp.tile([128, 2, 128], bf16, name=f"w2sb{t}")
                    nc.sync.dma_start(out=w2t_, in_=w2_d[t])
                    w2sb.append(w2t_)

            xts, s1ps, t0s = {}, {}, {}

            def stage_a(i):
                # load x(i); s1(i) = sign(x + b11) into padded bf16 tiles
                xts[i], s1ps[i] = [], []
                for ct in range(2):
                    xt = work.tile([128, H, W], f32, tag=f"xt{ct}", bufs=4,
                                   name=f"xt{ct}_{i}")
                    nc.sync.dma_start(out=xt, in_=x_d[i, ct * 128:(ct + 1) * 128])
                    sp = work.tile([128, 34, 34], bf16, tag=f"s1p{ct}", bufs=3,
                                   name=f"s1p{ct}_{i}")
                    nc.gpsimd.memset(sp[:, 0, :], 0.0)
                    nc.gpsimd.memset(sp[:, 33, :], 0.0)
                    nc.gpsimd.memset(sp[:, 1:33, 0], 0.0)
                    nc.gpsimd.memset(sp[:, 1:33, 33], 0.0)
                    nc.scalar.activation(sp[:, 1:33, 1:33], xt, AF.Sign,
                                         bias=cv[ct][:, 0:1])
                    xts[i].append(xt)
                    s1ps[i].append(sp)

            def stage_b(i):
                # conv1(i): 72 matmuls; t0 = A1*psum + C1 on ACT (frees PSUM)
                t0s[i] = []
                sp = s1ps[i]
                for oct in range(2):
                    t0 = work.tile([128, 1024], f32, tag=f"t0_{oct}", bufs=3,
                                   name=f"t0_{oct}_{i}")
                    for hh in range(2):
                        # one single-bank psum tile per (oct,hh): the t0 read
                        # of each bank unblocks the next image's matching conv
                        # group independently
                        p1t = ps.tile([128, 512], f32, tag=f"ps1_{oct}{hh}",
                                      bufs=1, name=f"ps1_{oct}{hh}_{i}")
                        first = True
                        for tap in range(9):
                            kh, kw = divmod(tap, 3)
                            for ict in range(2):
                                nc.tensor.matmul(
                                    p1t,
                                    lhsT=w1sb[ict][:, tap, oct, :],
                                    rhs=sp[ict][:, hh * 16 + kh:hh * 16 + kh + 16,
                                                kw:kw + 32],
                                    start=first, stop=(tap == 8 and ict == 1))
                                first = False
                        nc.scalar.activation(t0[:, hh * 512:(hh + 1) * 512],
                                             p1t, AF.Identity,
                                             bias=cv[oct][:, 2:3],
                                             scale=cv[oct][:, 1:2])
                    t0s[i].append(t0)

            def stage_c(i):
                # epilogue1 -> s2 -> conv2 -> epilogue2 -> store
                # prelu(t) = a1h*t + b1h*|t| with a1h=(1+a)/2, b1h=(1-a)/2>0
                s2 = []
                for ct in range(2):
                    t0 = t0s[i][ct]
                    xflat = xts[i][ct].rearrange("p a b -> p (a b)")
                    nc.vector.tensor_add(out=t0, in0=t0, in1=xflat)  # pre1
                    u = work.tile([128, 1024], f32, tag=f"u_{ct}", bufs=2,
                                  name=f"u_{ct}_{i}")
                    nc.scalar.activation(u, t0, AF.Abs, scale=cv[ct][:, 3:4])
                    nc.vector.tensor_scalar_mul(t0, t0, cv[ct][:, 4:5])
                    nc.vector.tensor_add(out=t0, in0=t0, in1=u)  # p1
                    s2t = work.tile([128, 1024], bf16, tag=f"s2_{ct}", bufs=3,
                                    name=f"s2_{ct}_{i}")
                    nc.scalar.activation(s2t, t0, AF.Sign, bias=cv[ct][:, 5:6])
                    s2.append(s2t)
                for oct in range(2):
                    p2t = ps.tile([128, 1024], f32, tag=f"ps2_{oct}", bufs=1,
                                  name=f"ps2_{oct}_{i}")
                    for hh in range(2):
                        for ict in range(2):
                            nc.tensor.matmul(
                                p2t[:, hh * 512:(hh + 1) * 512],
                                lhsT=w2sb[ict][:, oct, :],
                                rhs=s2[ict][:, hh * 512:(hh + 1) * 512],
                                start=(ict == 0), stop=(ict == 1))
                    t0b = work.tile([128, 1024], f32, tag=f"t0b_{oct}", bufs=2,
                                    name=f"t0b_{oct}_{i}")
                    nc.scalar.activation(t0b, p2t, AF.Identity,
                                         bias=cv[oct][:, 7:8],
                                         scale=cv[oct][:, 6:7])
                    nc.vector.tensor_add(out=t0b, in0=t0b, in1=t0s[i][oct])  # pre2
                    u2 = work.tile([128, 1024], f32, tag=f"u2_{oct}", bufs=2,
                                   name=f"u2_{oct}_{i}")
                    nc.scalar.activation(u2, t0b, AF.Abs, scale=cv[oct][:, 8:9])
                    nc.vector.tensor_scalar(t0b, t0b, cv[oct][:, 9:10],
                                            cv[oct][:, 10:11],
                                            op0=ALU.mult, op1=ALU.add)
                    nc.vector.tensor_add(out=t0b, in0=t0b, in1=u2)  # out2
                    nc.sync.dma_start(
                        out=out_d[i, oct * 128:(oct + 1) * 128],
                        in_=t0b.rearrange("p (a b) -> p a b", a=H))

            # PE warm-up: dummy matmuls on a zeroed tile run during the
            # startup DMA wait so conv1(0) starts at full HAM clock rate
            warm = work.tile([128, 512], bf16, name="warm")
            nc.gpsimd.memset(warm, 0.0)
            wps = ps.tile([128, 512], f32, tag="ps1_00", bufs=1, name="wps")
            for r in range(18):
                nc.tensor.matmul(wps, lhsT=warm[:, 0:128], rhs=warm,
                                 start=(r == 0), stop=(r == 17))

            for it in range(n_img + 2):
                if it < n_img:
                    stage_a(it)
                if it == 0:
                    load_weights()
                if 1 <= it <= n_img:
                    stage_b(it - 1)
                if 2 <= it:
                    stage_c(it - 2)

    nc.compile()
    return nc


def _prep_host(inputs):
    """Host-side O(C^2) weight/constant preprocessing (numpy)."""
    f = lambda k: np.asarray(inputs[k], dtype=np.float32)
    w1, w2 = f("w1"), f("w2")
    b11, b12, b13 = f("b11"), f("b12"), f("b13")
    b21, b22, b23 = f("b21"), f("b22"), f("b23")
    a1, a2 = f("a1"), f("a2")
    g1, be1, m1, v1 = f("g1m"), f("be1m"), f("m1m"), f("v1m")
    g2, be2, m2, v2 = f("g2m"), f("be2m"), f("m2m"), f("v2m")

    scale1 = np.abs(w1).mean(axis=(1, 2, 3), dtype=np.float64).astype(np.float32)
    scale2 = np.abs(w2).mean(axis=(1, 2, 3), dtype=np.float64).astype(np.float32)

    # sign(w1): [oc, ic, kh, kw] -> [ict, ic_lo, tap, oct, oc_lo]
    sgn1 = np.sign(w1).reshape(2, 128, 2, 128, 9).transpose(2, 3, 4, 0, 1)
    w1t = np.ascontiguousarray(sgn1).astype(ml_dtypes.bfloat16)
    # sign(w2): [oc, ic] -> [ict, ic_lo, oct, oc_lo]
    sgn2 = np.sign(w2).reshape(256, 256).reshape(2, 128, 2, 128)
    w2t = np.ascontiguousarray(sgn2.transpose(2, 3, 0, 1)).astype(ml_dtypes.bfloat16)

    inv1 = g1 / np.sqrt(v1 + EPS)
    inv2 = g2 / np.sqrt(v2 + EPS)
    A1 = scale1 * inv1
    C1 = be1 - m1 * inv1 + b12
    A2 = scale2 * inv2
    C2p = be2 - m2 * inv2 + b22 + b13
    cv = np.stack([
        b11, A1, C1, (1.0 - a1) / 2.0, (1.0 + a1) / 2.0, b13 + b21,
        A2, C2p, (1.0 - a2) / 2.0, (1.0 + a2) / 2.0, b23,
    ], axis=-1).astype(np.float32).reshape(2, 128, 11)
    return w1t, w2t, np.ascontiguousarray(cv)


def _make_runner(nc):
    """Persistent jitted 8-core executor (compiles once, reusable across
    kernel() calls). Mirrors bass2jax.run_bass_via_pjrt's multi-core path."""
    import jax
    from jax.experimental.shard_map import shard_map
    from jax.sharding import Mesh, PartitionSpec
    from concourse.bass2jax import (install_neuronx_cc_hook, _bass_exec_p,
                                    partition_id_tensor)

    install_neuronx_cc_hook()
    pname = nc.partition_id_tensor.name if nc.partition_id_tensor else None
    in_names, out_names, out_avals, zero_outs = [], [], [], []
    for alloc in nc.m.functions[0].allocations:
        if not isinstance(alloc, mybir.MemoryLocationSet):
            continue
        name = alloc.memorylocations[0].name
        if alloc.kind == "ExternalInput":
            if name != pname:
                in_names.append(name)
        elif alloc.kind == "ExternalOutput":
            out_names.append(name)
            shape = tuple(alloc.tensor_shape)
            dtype = mybir.dt.np(alloc.dtype)
            out_avals.append(jax.core.ShapedArray(shape, dtype))
            zero_outs.append(np.zeros(shape, dtype))
    all_names = in_names + out_names + ([pname] if pname else [])

    def _body(*args):
        operands = list(args)
        if pname is not None:
            operands.append(partition_id_tensor())
        return tuple(_bass_exec_p.bind(
            *operands, out_avals=tuple(out_avals), in_names=tuple(all_names),
            out_names=tuple(out_names), lowering_input_output_aliases=(),
            sim_require_finite=True, sim_require_nnan=True, nc=nc))

    devices = jax.devices()[:N_CORES]
    assert len(devices) == N_CORES
    mesh = Mesh(np.asarray(devices), ("core",))
    spec = PartitionSpec("core")
    n_args = len(in_names) + len(out_names)
    jitted = jax.jit(
        shard_map(_body, mesh=mesh, in_specs=(spec,) * n_args,
                  out_specs=(spec,) * len(out_names), check_rep=False),
        keep_unused=True,
    )

    def run(per_core_in):
        concat_in = [np.concatenate([m[nm] for m in per_core_in], axis=0)
                     for nm in in_names]
        concat_zeros = [np.zeros((N_CORES * z.shape[0], *z.shape[1:]), z.dtype)
                        for z in zero_outs]
        outs = jitted(*concat_in, *concat_zeros)
        oix = out_names.index("out")
        return np.asarray(outs[oix])  # [N_CORES*PIMG, C, H, W]

    return run


def kernel(**inputs):
    x = np.ascontiguousarray(np.asarray(inputs["x"], dtype=np.float32))
    w1t, w2t, cv = _prep_host(inputs)

    if "nc" not in _CACHE:
        _CACHE["nc"] = _build_program(PIMG)
    nc = _CACHE["nc"]

    in_maps = [{
        "x": x[c * PIMG:(c + 1) * PIMG],
        "w1t": w1t,
        "w2t": w2t,
        "cv": cv,
    } for c in range(N_CORES)]

    try:
        if "runner" not in _CACHE:
            _CACHE["runner"] = _make_runner(nc)
        return _CACHE["runner"](in_maps)
    except Exception:
        _CACHE.pop("runner", None)
        res = run_bass_kernel_spmd(nc, in_maps, core_ids=list(range(N_CORES)))
        return np.concatenate([r["out"] for r in res.results], axis=0)

